# revision 15
# baseline (speedup 1.0000x reference)
"""Trainium2 Bass kernel for nn_ConsciousnessMetrics_57715770524288.

Reference math (see problem reference):
    d_eeg  = min(mean_row_entropy(psi) * mean_row_std(psi) * 3, 10)
    h_fmri = min(mean_row_norm(field) * |mean adj-col corr(field)| * 2, 5)
    clz    = min(pair_histogram_entropy(psi) + 0.3 * std(field), 3)
    out    = clip(w0*d_eeg/10 + w1*h_fmri/5 + w2*clz/3, 0, 1)

For the specified input distributions (psi ~ U[0,1), field ~ N(0,1)):
  - d_eeg's raw value is ~887 (clip at 10, margin ~88x)  -> d_eeg = 10.0
  - clz's raw value is >= ~4.3 (clip at 3, margin >=1.4x) -> clz  = 3.0
Both are verified at runtime from a row subsample of psi (plus the exact
field std computed on device); if the margins do not hold, we fall back to
an exact host computation. The only data-dependent quantity in the hot
path is h_fmri, which is computed exactly on the 8 NeuronCores.

Device strategy (data-parallel over the batch dim, 1024 rows/core):
  Only `fractal_field` is read (psi influences only clipped-away terms).
  Per core, 8 row-tiles [128, 4096] are loaded into SBUF with a ones
  column interleaved every 128 columns. One fp32 matmul per 128-column
  group per row-tile computes, via PSUM accumulation over row-tiles:
      out[0, n]     = sum_rows field[:, c0+n]                 (S1)
      out[j+1, j]   = sum_rows field[:, c0+j]^2               (S2)
      out[j+1, j+1] = sum_rows field[:, c0+j]*field[:, c0+j+1](S11)
  The 32 accumulator blocks [128, 128] exactly fill the 8 PSUM banks.
  ScalarE computes per-row sum-of-squares (Square activation with
  accum_out) for the row norms. Host sums the tiny per-core partials,
  fills in the 63 group-boundary S2/S11 values directly from the input,
  and finishes the correlation/norm/final-scalar math in float64.
"""

import numpy as np

B, E = 8192, 4096
NCORES = 8
ROWS_PER_CORE = B // NCORES          # 1024
TILES_PER_CORE = ROWS_PER_CORE // 128  # 8
G = E // 128                          # 32 column groups
GW = 129                              # group width in SBUF (ones col + 128 field cols)

D_EEG_MAX, H_FMRI_MAX, CLZ_MAX, D_MAX, N_LEVELS = 10.0, 5.0, 3.0, 1.0, 8

_NC = None            # compiled bass module (built once)
TRACE = False         # set True (e.g. from test.py) to capture a HW profile
LAST_EXEC_NS = None   # exec_time_ns from the last traced run
LAST_TRACE_PATH = None
LAST_DEBUG = {}       # host-side partials for validation


def _build():
    from contextlib import ExitStack

    import concourse.bacc as bacc
    import concourse.mybir as mybir
    import concourse.tile as tile

    nc = bacc.Bacc(
        "TRN2", target_bir_lowering=False, debug=False, num_devices=NCORES
    )
    # float32r end-to-end for the matmul path: same 32-bit layout as f32,
    # but the BIR verifier requires the producer (the DMA) of an FP32r
    # matmul operand to be FP32r itself.
    # Host pre-interleaves a ones column before every 128 field columns
    # (lhsT = [ones | F cols] must be contiguous), so loads are one fully
    # contiguous DMA per row-tile.
    field = nc.dram_tensor(
        "field", [ROWS_PER_CORE, G * GW], mybir.dt.float32r, kind="ExternalInput"
    )
    gram = nc.dram_tensor(
        "gram", [2, 128, 2048], mybir.dt.float32, kind="ExternalOutput"
    )
    rs = nc.dram_tensor(
        "rs", [128, TILES_PER_CORE], mybir.dt.float32, kind="ExternalOutput"
    )


    fld = field.ap()
    with tile.TileContext(nc) as tc, ExitStack() as ctx:
        tpool = ctx.enter_context(tc.tile_pool(name="tiles", bufs=8))
        spool = ctx.enter_context(tc.tile_pool(name="scratch", bufs=1))
        ppool = ctx.enter_context(tc.tile_pool(name="acc", bufs=1, space="PSUM"))
        opool = ctx.enter_context(tc.tile_pool(name="outs", bufs=1))

        rs_t = opool.tile([128, TILES_PER_CORE], mybir.dt.float32, tag="rs", name="rs_t")
        # Persistent SBUF accumulators, one per PSUM half (4 banks each).
        acc = [
            opool.tile([128, 2048], mybir.dt.float32, tag=f"acc{h}", name=f"acc{h}")
            for h in range(2)
        ]
        f32r = mybir.dt.float32r

        for t in range(TILES_PER_CORE):
            tl = tpool.tile([128, G * GW], mybir.dt.float32r, tag="ftile", name=f"ftile{t}")
            t3 = tl[:].rearrange("p (g c) -> p g c", c=GW)
            nc.sync.dma_start(tl[:], fld[t * 128 : (t + 1) * 128, :])
            # Per-tile partial Gram blocks, float32r (1 cyc/row needs N>=256).
            # Each matmul is its own accumulation group (start&stop=True ->
            # pure overwrite). Within a bank, four 128-col blocks are laid
            # down by chained 256-wide writes at offsets 0/128/256 plus a
            # 128-wide write at 384: each write's garbage half is overwritten
            # by the next (WAW deps keep the order). Cross-tile accumulation
            # happens in SBUF on the vector engine, per PSUM half, ping-pong.
            for half in range(2):
                pp = ppool.tile(
                    [128, 2048], mybir.dt.float32, tag=f"pp{half}", name=f"pp{half}_{t}"
                )
                for h in range(16):
                    g = 16 * half + h
                    b, s = divmod(h, 4)
                    n = 128 if s == 3 else 256
                    nc.tensor.matmul(
                        pp[:, 512 * b + 128 * s : 512 * b + 128 * s + n],
                        lhsT=tl[:, GW * g : GW * g + 128],
                        rhs=tl[:, GW * g + 1 : GW * g + 1 + n],
                        start=True,
                        stop=True,
                    )
                if t == 0:
                    nc.vector.tensor_copy(acc[half][:], pp[:])
                else:
                    nc.vector.tensor_add(acc[half][:], pp[:], acc[half][:])
            sc = spool.tile([128, G * 128], mybir.dt.float32, tag="sq", name=f"sq{t}")
            nc.scalar.activation(
                sc[:].rearrange("p (g c) -> p g c", c=128),
                t3[:, :, 1:GW].bitcast(mybir.dt.float32),
                mybir.ActivationFunctionType.Square,
                accum_out=rs_t[:, t : t + 1],
            )
        for h in range(2):
            nc.sync.dma_start(gram.ap()[h], acc[h][:])
        nc.sync.dma_start(rs.ap()[:], rs_t[:])
    nc.compile()
    return nc


def _enable_axon_ntff_hook():
    """Register the NTFF profiling hook (the image's antenv lacks
    axon_hooks, so trace=True would otherwise be unavailable)."""
    import sys
    import types

    try:
        from antenv.axon_hooks import get_axon_ntff_profile_hook  # noqa: F401

        return
    except ImportError:
        pass
    import antenv

    mod = types.ModuleType("antenv.axon_hooks")
    mod._hook = None
    mod.set_axon_ntff_profile_hook = lambda h: setattr(mod, "_hook", h)
    mod.get_axon_ntff_profile_hook = lambda: mod._hook
    sys.modules["antenv.axon_hooks"] = mod
    antenv.axon_hooks = mod
    from trn_agent_boot.trn_boot import _ntff_profile_via_ctypes

    mod.set_axon_ntff_profile_hook(
        _ntff_profile_via_ctypes("/opt/axon/libaxon_pjrt.so")
    )
    import concourse.bass_utils as bu

    bu.upload_artifacts = lambda tmpdir: tmpdir  # no artifact bucket here


def _run_device(field_np):
    global _NC, LAST_EXEC_NS, LAST_TRACE_PATH
    from concourse.bass_utils import run_bass_kernel_spmd

    if TRACE:
        _enable_axon_ntff_hook()
    if _NC is None:
        _NC = _build()
    inter = np.ones((B, G, GW), np.float32)
    inter[:, :, 1:] = field_np.reshape(B, G, 128)
    inter = inter.reshape(B, G * GW)
    in_maps = [
        {"field": inter[i * ROWS_PER_CORE : (i + 1) * ROWS_PER_CORE]}
        for i in range(NCORES)
    ]
    res = run_bass_kernel_spmd(_NC, in_maps, list(range(NCORES)), trace=TRACE)
    if res.exec_time_ns is not None:
        LAST_EXEC_NS = res.exec_time_ns
    if res.instructions_and_trace is not None:
        LAST_TRACE_PATH = res.instructions_and_trace[1]
    gram_sum = np.zeros((2, 128, 2048), np.float64)
    rs_all = np.empty((NCORES, 128, TILES_PER_CORE), np.float64)
    for i in range(NCORES):
        gram_sum += res.results[i]["gram"].astype(np.float64)
        rs_all[i] = res.results[i]["rs"].astype(np.float64)
    return gram_sum, rs_all


def _host_exact(psi, field, w):
    """Exact float64 mirror of the reference (fallback path)."""
    psi64 = psi.astype(np.float64)
    f = field.astype(np.float64)
    ent = -(psi64 * np.log(psi64 + 1e-10)).sum(-1).mean()
    sv = psi64.std(-1, ddof=1).mean()
    d_eeg = min(ent * sv * 3.0, D_EEG_MAX)

    h_fmri = _h_fmri_from_stats(*_field_stats_host(f), f)

    q = np.clip(np.floor(psi * np.float32(N_LEVELS)), 0, N_LEVELS - 1).astype(np.int64)
    pair = (q[:, :-1] * N_LEVELS + q[:, 1:]).ravel()
    counts = np.bincount(pair, minlength=N_LEVELS * N_LEVELS).astype(np.float64)
    p = counts / pair.size
    cond_ent = -(p[p > 0] * np.log2(p[p > 0])).sum()
    fstd = f.std(ddof=1)
    clz = min(cond_ent + 0.3 * fstd, CLZ_MAX)
    return _combine(w, d_eeg, h_fmri, clz)


def _field_stats_host(f):
    S1 = f.sum(0)
    S2 = (f * f).sum(0)
    S11 = (f[:, :-1] * f[:, 1:]).sum(0)
    norm_mean = np.sqrt((f * f).sum(-1)).mean()
    return S1, S2, S11, norm_mean


def _h_fmri_from_stats(S1, S2, S11, norm_mean, _f=None):
    mean = S1 / B
    var = S2 - B * mean * mean
    cov = S11 - B * mean[:-1] * mean[1:]
    with np.errstate(invalid="ignore", divide="ignore"):
        corr = cov / np.sqrt(var[:-1] * var[1:])
    mask = ~np.isnan(corr)
    n = int(mask.sum())
    mean_corr = float(np.where(mask, corr, 0.0).sum() / max(n, 1)) if n > 0 else 0.0
    LAST_DEBUG.update(
        S1=S1, S2=S2, S11=S11, norm_mean=norm_mean, mean_corr=mean_corr
    )
    return min(norm_mean * abs(mean_corr) * 2.0, H_FMRI_MAX)


def _combine(w, d_eeg, h_fmri, clz):
    w = w.astype(np.float64)
    fci = (
        w[0] * (d_eeg / D_EEG_MAX)
        + w[1] * (h_fmri / H_FMRI_MAX)
        + w[2] * (clz / CLZ_MAX)
    )
    LAST_DEBUG.update(d_eeg=d_eeg, h_fmri=h_fmri, clz=clz)
    return np.array(np.clip(fci / D_MAX, 0.0, 1.0), dtype=np.float32)


def kernel(psi_distribution, fractal_field, fci_weights):
    psi = np.asarray(psi_distribution, dtype=np.float32)
    field = np.asarray(fractal_field, dtype=np.float32)
    w = np.asarray(fci_weights, dtype=np.float32)

    gram_sum, rs_all = _run_device(field)

    # Unpack per-group blocks: gram_sum[h][m, 128*(g%16)+n] -> blocks[g, m, n]
    blocks = (
        gram_sum.reshape(2, 128, 16, 128).transpose(0, 2, 1, 3).reshape(G, 128, 128)
    )
    j = np.arange(127)
    S1 = blocks[:, 0, :].reshape(E)
    S2 = np.empty(E, np.float64)
    S11e = np.empty(E, np.float64)  # S11e[c] = sum field[:,c]*field[:,c+1]
    S2.reshape(G, 128)[:, :127] = blocks[:, j + 1, j]
    S11e.reshape(G, 128)[:, :127] = blocks[:, j + 1, j + 1]
    # group-boundary columns c = 128g+127 directly from the input (63 sums)
    f64 = field.astype(np.float64)
    bcols = 128 * np.arange(G) + 127
    S2[bcols] = (f64[:, bcols] ** 2).sum(0)
    lcols = bcols[:-1]
    S11e[lcols] = (f64[:, lcols] * f64[:, lcols + 1]).sum(0)
    S11 = S11e[: E - 1]

    norm_mean = float(np.sqrt(rs_all).mean())
    h_fmri = _h_fmri_from_stats(S1, S2, S11, norm_mean)

    # d_eeg / clz clip with wide margins for the specified input
    # distributions; verify from a row subsample + exact field std.
    tot_sum = S1.sum()
    tot_sumsq = S2.sum()
    nel = B * E
    fstd = np.sqrt(max(tot_sumsq - tot_sum * tot_sum / nel, 0.0) / (nel - 1))
    psub = psi[::16]
    psub64 = psub.astype(np.float64)
    ent = -(psub64 * np.log(psub64 + 1e-10)).sum(-1).mean()
    sv = psub64.std(-1, ddof=1).mean()
    d_raw = ent * sv * 3.0
    q = np.clip(np.floor(psub * np.float32(N_LEVELS)), 0, N_LEVELS - 1).astype(np.int64)
    pair = (q[:, :-1] * N_LEVELS + q[:, 1:]).ravel()
    counts = np.bincount(pair, minlength=N_LEVELS * N_LEVELS).astype(np.float64)
    p = counts / pair.size
    cond_ent_est = -(p[p > 0] * np.log2(p[p > 0])).sum()
    LAST_DEBUG.update(d_raw_est=d_raw, clz_raw_est=cond_ent_est + 0.3 * fstd, fstd=fstd)
    if d_raw < 2.0 * D_EEG_MAX or cond_ent_est + 0.3 * fstd < 1.15 * CLZ_MAX:
        return _host_exact(psi, field, w)

    return _combine(w, D_EEG_MAX, h_fmri, CLZ_MAX)
